# revision 27
# baseline (speedup 1.0000x reference)
"""Trainium2 Bass kernel for a dense transformer block (pre-LN MHA + MLP).

Sharding: pure data parallel — batch (8) maps 1:1 onto the 8 NeuronCores.
Each core runs the full block on one [1024, 1024] slice with replicated
weights (host-cast to fp16 for the tensor engine; fp32 residual path).

Self-contained: hardcodes all shapes from the problem spec.
"""

from contextlib import ExitStack

import numpy as np

import concourse.bass as bass
import concourse.tile as tile
from concourse import bacc, mybir
from concourse.bass import ts
from concourse.bass_utils import run_bass_kernel_spmd
from concourse.masks import make_identity

F32 = mybir.dt.float32
F16 = mybir.dt.float16
AF = mybir.ActivationFunctionType
ALU = mybir.AluOpType

P = 128          # partitions
N = 1024         # tokens per core
D = 1024         # model dim
KC = D // P      # 8 contraction chunks of 128
HEADS = 16
HD = 64          # head dim
HID = 4096
EPS = 1e-6
NT = N // 512    # 2 chunks of 512 tokens
MT = N // P      # 8 token tiles of 128
SCALE = HD ** -0.5


def build_block(ln1_triv, ln2_triv, qk_triv, apply_c1, apply_bfc2):
    nc = bacc.Bacc("TRN2", target_bir_lowering=False, debug=False, num_devices=8)

    x_d = nc.dram_tensor("x", [N, D], F32, kind="ExternalInput")
    wqkv_d = nc.dram_tensor("w_qkv", [D, 3 * D], F16, kind="ExternalInput")
    wproj_d = nc.dram_tensor("w_proj", [D, D], F16, kind="ExternalInput")
    wfc1_d = nc.dram_tensor("w_fc1", [D, HID], F16, kind="ExternalInput")
    wfc2_d = nc.dram_tensor("w_fc2", [HID, D], F16, kind="ExternalInput")
    bqkv_d = nc.dram_tensor("b_qkv", [3 * D], F32, kind="ExternalInput")
    bfc1_d = nc.dram_tensor("b_fc1", [HID], F32, kind="ExternalInput")
    ln1s_d = nc.dram_tensor("ln1_scale", [D], F32, kind="ExternalInput")
    ln1b_d = nc.dram_tensor("ln1_bias", [D], F32, kind="ExternalInput")
    ln2s_d = nc.dram_tensor("ln2_scale", [D], F32, kind="ExternalInput")
    ln2b_d = nc.dram_tensor("ln2_bias", [D], F32, kind="ExternalInput")
    c1_d = nc.dram_tensor("c1", [D], F32, kind="ExternalInput") if apply_c1 else None
    bfc2_d = (
        nc.dram_tensor("b_fc2c", [D], F32, kind="ExternalInput") if apply_bfc2 else None
    )
    y_d = nc.dram_tensor("y", [N, D], F16, kind="ExternalOutput")

    # [(kc p), n] -> [p, kc, n] views for weight loads (lhsT layout)
    wqkv_v = wqkv_d.ap().rearrange("(kc p) n -> p kc n", p=P)
    wproj_v = wproj_d.ap().rearrange("(kc p) n -> p kc n", p=P)
    wfc1_v = wfc1_d.ap().rearrange("(kc p) n -> p kc n", p=P)
    wfc2_v = wfc2_d.ap().rearrange("(kc p) n -> p kc n", p=P)

    with tile.TileContext(nc) as tc, ExitStack() as ctx:
        ep = ctx.enter_context
        constp = ep(tc.tile_pool(name="const", bufs=1))
        xload = ep(tc.tile_pool(name="xload", bufs=2))
        x1p = ep(tc.tile_pool(name="x1", bufs=1))
        htmpp = ep(tc.tile_pool(name="htmp", bufs=2))
        hTp = ep(tc.tile_pool(name="hT", bufs=1))
        qTp = ep(tc.tile_pool(name="qT", bufs=1))
        kTp = ep(tc.tile_pool(name="kT", bufs=1))
        vp = ep(tc.tile_pool(name="vv", bufs=1))
        oTp = ep(tc.tile_pool(name="oT", bufs=1))
        probsp = ep(tc.tile_pool(name="probs", bufs=3))
        wp = ep(tc.tile_pool(name="w", bufs=4))
        statsp = ep(tc.tile_pool(name="stats", bufs=4))
        otmpp = ep(tc.tile_pool(name="otmp", bufs=1))
        pmp = ep(tc.tile_pool(name="pm", bufs=2, space="PSUM"))
        pvp = ep(tc.tile_pool(name="pv", bufs=1, space="PSUM"))
        ptp = ep(tc.tile_pool(name="pt", bufs=2, space="PSUM"))

        # ---- first x tile load goes out before anything else ----
        # two half-DMAs so bn_stats on the first half starts sooner
        x_t0 = xload.tile([P, D], F32, tag="x_t")
        nc.sync.dma_start(x_t0[:, 0:512], x_d.ap()[ts(0, P), 0:512])
        nc.sync.dma_start(x_t0[:, 512:1024], x_d.ap()[ts(0, P), 512:1024])

        # ---- constants (gpsimd queue; keeps sync queue on x) ----
        ident = constp.tile([P, P], F16)
        make_identity(nc, ident[:])
        eps_t = constp.tile([P, 1], F32)
        nc.vector.memset(eps_t[:], EPS)
        ones_t = constp.tile([P, HD], F16)
        nc.vector.memset(ones_t[:], 1.0)
        ln1s = constp.tile([P, KC], F32)
        nc.gpsimd.dma_start(ln1s[:], ln1s_d.ap().rearrange("(k p) -> p k", p=P))
        ln1b = constp.tile([P, KC], F32)
        nc.gpsimd.dma_start(ln1b[:], ln1b_d.ap().rearrange("(k p) -> p k", p=P))
        ln2s = constp.tile([P, KC], F32)
        nc.gpsimd.dma_start(ln2s[:], ln2s_d.ap().rearrange("(k p) -> p k", p=P))
        ln2b = constp.tile([P, KC], F32)
        nc.gpsimd.dma_start(ln2b[:], ln2b_d.ap().rearrange("(k p) -> p k", p=P))
        bqk = constp.tile([P, 16], F32)  # q,k bias columns (out_c 0..2047)
        bqkv_v = bqkv_d.ap().rearrange("(m p) -> p m", p=P)
        nc.gpsimd.dma_start(bqk[:], bqkv_v[:, 0:16])
        bfc1 = constp.tile([P, HID // P], F32)
        nc.gpsimd.dma_start(bfc1[:], bfc1_d.ap().rearrange("(m p) -> p m", p=P))
        if apply_c1:
            c1row = constp.tile([P, D], F32)
            src = c1_d.ap()
            nc.gpsimd.dma_start(
                c1row[:],
                bass.AP(tensor=src.tensor, offset=src.offset, ap=[[0, P], [1, D]]),
            )
        if apply_bfc2:
            b2row = constp.tile([P, D], F32)
            src = bfc2_d.ap()
            nc.gpsimd.dma_start(
                b2row[:],
                bass.AP(tensor=src.tensor, offset=src.offset, ap=[[0, P], [1, D]]),
            )

        hT = hTp.tile([P, KC, N], F16, tag="hT")

        def layer_norm_to_hT(src_ap, out_hT, s_cols, b_cols, mt, trivial):
            """LN over free dim of src [128, 1024]; write transposed fp16 into
            out_hT[:, kc, mt*128:...]. Work split across DVE/GPSIMD/ACT."""
            st = statsp.tile([P, 2, 6], F32)
            xr = src_ap.rearrange("p (a b) -> p a b", b=512)
            nc.vector.bn_stats(st[:, 0, :], xr[:, 0, :])
            nc.vector.bn_stats(st[:, 1, :], xr[:, 1, :])
            mv = statsp.tile([P, 2], F32)
            nc.vector.bn_aggr(mv[:], st[:])
            rstd = statsp.tile([P, 1], F32)
            nc.scalar.activation(rstd[:], mv[:, 1:2], AF.Sqrt, bias=eps_t[:])
            nc.vector.reciprocal(rstd[:], rstd[:])
            h = htmpp.tile([P, D], F16)
            nc.vector.tensor_scalar(
                out=h[:, 0:512], in0=src_ap[:, 0:512], scalar1=mv[:, 0:1],
                scalar2=rstd[:], op0=ALU.subtract, op1=ALU.mult,
            )
            nc.gpsimd.tensor_scalar(
                out=h[:, 512:1024], in0=src_ap[:, 512:1024], scalar1=mv[:, 0:1],
                scalar2=rstd[:], op0=ALU.subtract, op1=ALU.mult,
            )
            for kc in range(KC):
                pt_t = ptp.tile([P, P], F16, tag="pt")
                nc.tensor.transpose(pt_t[:], h[:, ts(kc, P)], ident[:])
                dst = out_hT[:, kc, ts(mt, P)]
                if trivial:
                    nc.scalar.copy(dst, pt_t[:])
                else:
                    nc.vector.tensor_scalar(
                        out=dst, in0=pt_t[:],
                        scalar1=s_cols[:, kc : kc + 1], scalar2=b_cols[:, kc : kc + 1],
                        op0=ALU.mult, op1=ALU.add,
                    )

        # ---- phase 1: LN1 + transpose ----
        for mt in range(MT):
            if mt == 0:
                x_t = x_t0
            else:
                x_t = xload.tile([P, D], F32, tag="x_t")
                nc.sync.dma_start(x_t[:, 0:512], x_d.ap()[ts(mt, P), 0:512])
                nc.sync.dma_start(x_t[:, 512:1024], x_d.ap()[ts(mt, P), 512:1024])
            layer_norm_to_hT(x_t[:], hT, ln1s, ln1b, mt, ln1_triv)

        # ---- phase 2: qkv (nt-outer so chains start once half of hT is up) --
        qT = qTp.tile([P, KC, N], F16, tag="qT")
        kT = kTp.tile([P, KC, N], F16, tag="kT")
        v_sb = vp.tile([P, MT, HEADS * (HD + 1)], F16, tag="vv")

        def wpiece(view, n0):
            t = wp.tile([P, KC, 512], F16, tag="w")
            nc.sync.dma_start(t[:], view[:, :, n0 : n0 + 512])
            return t

        for half in range(2):  # 0: q (cols 0:1024), 1: k (cols 1024:2048)
            pieces = [wpiece(wqkv_v, half * 1024), wpiece(wqkv_v, half * 1024 + 512)]
            dst_t = qT if half == 0 else kT
            for nt in range(NT):
                for mc_l in range(8):
                    mc = half * 8 + mc_l
                    piece = pieces[mc_l // 4]
                    ps = pmp.tile([P, 512], F32, tag="pm")
                    for kc in range(KC):
                        nc.tensor.matmul(
                            ps[:], piece[:, kc, ts(mc_l % 4, P)],
                            hT[:, kc, ts(nt, 512)],
                            start=(kc == 0), stop=(kc == KC - 1),
                        )
                    dst = dst_t[:, mc_l, ts(nt, 512)]
                    if qk_triv:
                        if mc_l % 2 == 0:
                            nc.vector.tensor_copy(dst, ps[:])
                        else:
                            nc.scalar.copy(dst, ps[:])
                    else:
                        nc.vector.tensor_scalar(
                            out=dst, in0=ps[:], scalar1=bqk[:, mc : mc + 1],
                            scalar2=None, op0=ALU.add,
                        )

        # ---- attention emission helpers ----
        def scores_pair(h, probs, mk):
            """one [128,1024] scoresT stripe + exp into probs[:, mk, :]."""
            mc_h = h // 2
            pr = (h % 2) * HD
            ps = pmp.tile([P, N], F32, tag="pm")
            for nq in range(NT):
                nc.tensor.matmul(
                    ps[:, ts(nq, 512)],
                    kT[pr : pr + HD, mc_h, ts(mk, P)],
                    qT[pr : pr + HD, mc_h, ts(nq, 512)],
                    start=True, stop=True,
                )
            nc.scalar.activation(probs[:, mk, :], ps[:], AF.Exp, scale=SCALE)

        def scores_group(h):
            probs = probsp.tile([P, KC, N], F16, tag="probs")
            for mk in range(MT):
                scores_pair(h, probs, mk)
            return probs

        oT = oTp.tile([P, KC, N], F16, tag="oT")

        # v (token-major, ones column per head at stride 65), interleaved with
        # the first two heads' score stripes so their exps overlap v matmuls
        v_pieces = [wpiece(wqkv_v, n0) for n0 in (2048, 2560)]
        # odd heads first: even heads (base partition 0) can write oT without
        # a shifting DMA, so the last-processed heads retire fastest
        HORD = [h for h in range(HEADS) if h % 2] + [h for h in range(HEADS) if not h % 2]
        probs_n_alloc = [0]

        def probs_tile():
            # every 4th tile borrows the hT slot (hT is dead once v is built;
            # the first borrowed tile is only written deep into attention) —
            # an effective 4-deep probs rotation
            i = probs_n_alloc[0]
            probs_n_alloc[0] += 1
            if i % 4 == 3:
                pb = hTp.tile([P, KC, N], F16, tag="hT")
            else:
                pb = probsp.tile([P, KC, N], F16, tag="probs")
            return pb

        probs_q = [probs_tile(), probs_tile()]
        for mt in range(MT):
            v_row = v_sb[:, mt, :].rearrange("p (h c) -> p h c", c=HD + 1)
            nc.vector.memset(v_row[:, :, HD : HD + 1], 1.0)
            ps = pmp.tile([P, N], F32, tag="pm")
            for nv in range(2):
                for kc in range(KC):
                    nc.tensor.matmul(
                        ps[:, ts(nv, 512)], hT[:, kc, ts(mt, P)],
                        v_pieces[nv][:, kc, :],
                        start=(kc == 0), stop=(kc == KC - 1),
                    )
            dst = v_row[:, :, 0:HD]
            src = ps[:].rearrange("p (h c) -> p h c", c=HD)
            if mt % 2 == 0:
                nc.vector.tensor_copy(dst, src)
            else:
                nc.scalar.copy(dst, src)
            # first two heads' score stripes ride the (here idle) pt pool so
            # the v chains keep both pm slots
            for hh in range(2):
                h0 = HORD[hh]
                mc0 = h0 // 2
                pr0 = (h0 % 2) * HD
                for nq in range(NT):
                    sps = ptp.tile([P, 512], F32, tag="pt")
                    nc.tensor.matmul(
                        sps[:],
                        kT[pr0 : pr0 + HD, mc0, ts(mt, P)],
                        qT[pr0 : pr0 + HD, mc0, ts(nq, 512)],
                        start=True, stop=True,
                    )
                    nc.scalar.activation(
                        probs_q[hh][:, mt, ts(nq, 512)], sps[:], AF.Exp, scale=SCALE
                    )

        # w_proj load early (streams behind attention)
        proj_pieces = [wpiece(wproj_v, n0) for n0 in (0, 512)]

        # ---- phase 3: attention main loop ----
        for hi, h in enumerate(HORD):
            probs_h = probs_q.pop(0)
            if hi + 2 < HEADS:
                probs_next = probs_tile()
                probs_q.append(probs_next)
                todo = list(range(MT))
                h_next = HORD[hi + 2]
            else:
                probs_next, todo, h_next = None, [], None
            mc_h = h // 2
            pr = (h % 2) * HD
            pav = pvp.tile([P, N], F32, tag="pv")
            for j, (nq, mk) in enumerate([(a, b) for a in range(NT) for b in range(MT)]):
                nc.tensor.matmul(
                    pav[0 : HD + 1, ts(nq, 512)],
                    v_sb[:, mk, h * (HD + 1) : (h + 1) * (HD + 1)],
                    probs_h[:, mk, ts(nq, 512)],
                    start=(mk == 0), stop=(mk == MT - 1),
                    skip_group_check=True,
                )
                if j % 2 == 1 and j < 12 and todo:
                    scores_pair(h_next, probs_next, todo.pop(0))
            # last two score stripes land here so PE stays busy while the
            # drain copy below holds the single pv slot
            while todo:
                scores_pair(h_next, probs_next, todo.pop(0))
            # drain psum promptly (frees the slot for the next head)
            av_f = otmpp.tile([HD + 1, N], F32, tag="av_f")
            nc.vector.tensor_copy(av_f[:], pav[0 : HD + 1, :])
            nc.vector.reciprocal(av_f[HD : HD + 1, :], av_f[HD : HD + 1, :])
            srow16 = otmpp.tile([1, N], F16, tag="srow16")
            nc.vector.tensor_copy(srow16[:], av_f[HD : HD + 1, :])
            if pr == 0:
                for nq in range(NT):
                    rb = ptp.tile([HD, 512], F32, tag="pt")
                    nc.tensor.matmul(
                        rb[:], ones_t[0:1, 0:HD], srow16[:, ts(nq, 512)],
                        start=True, stop=True,
                    )
                    nc.vector.tensor_mul(
                        oT[0:HD, mc_h, ts(nq, 512)], av_f[0:HD, ts(nq, 512)], rb[:]
                    )
            else:
                o_t = otmpp.tile([HD, N], F16, tag="o_t")
                for nq in range(NT):
                    rb = ptp.tile([HD, 512], F32, tag="pt")
                    nc.tensor.matmul(
                        rb[:], ones_t[0:1, 0:HD], srow16[:, ts(nq, 512)],
                        start=True, stop=True,
                    )
                    nc.vector.tensor_mul(
                        o_t[:, ts(nq, 512)], av_f[0:HD, ts(nq, 512)], rb[:]
                    )
                nc.sync.dma_start(oT[pr : pr + HD, mc_h, :], o_t[:])

        # ---- phase 4: proj + residual -> x1 ----
        x1 = x1p.tile([P, MT, D], F16)
        for mt in range(MT):
            x_t = xload.tile([P, D], F32, tag="x_t")
            nc.sync.dma_start(x_t[:], x_d.ap()[ts(mt, P), :])
            ps = pmp.tile([P, N], F32, tag="pm")
            for np_ in range(NT):
                for kc in range(KC):
                    nc.tensor.matmul(
                        ps[:, ts(np_, 512)], oT[:, kc, ts(mt, P)],
                        proj_pieces[np_][:, kc, :],
                        start=(kc == 0), stop=(kc == KC - 1),
                    )
            nc.vector.tensor_add(x1[:, mt, :], ps[:], x_t[:])
            if apply_c1:
                nc.vector.tensor_add(x1[:, mt, :], x1[:, mt, :], c1row[:])

        # ---- phase 5: LN2 + transpose ----
        h2T = hTp.tile([P, KC, N], F16, tag="hT")
        for mt in range(MT):
            layer_norm_to_hT(x1[:, mt, :], h2T, ln2s, ln2b, mt, ln2_triv)

        # ---- phase 6: fc1 (gelu) ----
        # a1T groups g=0..3 each [128, 8, 1024] fp16, reusing attention pools
        a1_pools = [(qTp, "qT"), (kTp, "kT"), (vp, "vv"), (oTp, "oT")]
        a1 = []
        for pool, tag in a1_pools:
            a1_g = pool.tile([P, KC, N], F16, tag=tag)
            a1.append(a1_g)
        for p8 in range(8):  # 512-wide hidden column pieces
            w1_t = wpiece(wfc1_v, p8 * 512)
            for nt in range(NT):
                for mh_l in range(4):
                    mhg = p8 * 4 + mh_l
                    ps = pmp.tile([P, 512], F32, tag="pm")
                    for kc in range(KC):
                        nc.tensor.matmul(
                            ps[:], w1_t[:, kc, ts(mh_l, P)], h2T[:, kc, ts(nt, 512)],
                            start=(kc == 0), stop=(kc == KC - 1),
                        )
                    nc.scalar.activation(
                        a1[mhg // 8][:, mhg % 8, ts(nt, 512)], ps[:],
                        AF.Gelu_apprx_tanh, bias=bfc1[:, mhg : mhg + 1],
                    )

        # ---- phase 7: fc2 + residual -> y ----
        # stream w2 in half-group pieces; accumulate partials into x1 per group
        for g in range(4):
            wa = wp.tile([P, 4, N], F16, tag="w")
            nc.sync.dma_start(wa[:], wfc2_v[:, g * 8 : g * 8 + 4, :])
            wb = wp.tile([P, 4, N], F16, tag="w")
            nc.sync.dma_start(wb[:], wfc2_v[:, g * 8 + 4 : g * 8 + 8, :])
            for mt in range(MT):
                ps = pmp.tile([P, N], F32, tag="pm")
                for ncol in range(NT):
                    for kc in range(KC):
                        w2 = wa if kc < 4 else wb
                        nc.tensor.matmul(
                            ps[:, ts(ncol, 512)], a1[g][:, kc, ts(mt, P)],
                            w2[:, kc % 4, ts(ncol, 512)],
                            start=(kc == 0), stop=(kc == KC - 1),
                        )
                nc.vector.tensor_add(x1[:, mt, :], ps[:], x1[:, mt, :])
        for mt in range(MT):
            if apply_bfc2:
                nc.vector.tensor_add(x1[:, mt, :], x1[:, mt, :], b2row[:])
            nc.sync.dma_start(y_d.ap()[ts(mt, P), :], x1[:, mt, :])

    nc.compile()
    return nc


_cache = {}


def _get_nc(*key):
    if key not in _cache:
        _cache[key] = build_block(*key)
    return _cache[key]


def kernel(
    x, w_qkv, b_qkv, w_proj, b_proj, ln1_scale, ln1_bias,
    ln2_scale, ln2_bias, w_fc1, b_fc1, w_fc2, b_fc2,
):
    x = np.asarray(x, np.float32)
    B = x.shape[0]
    b_qkv = np.asarray(b_qkv, np.float32)
    b_v = b_qkv[2 * D :]
    # exact folds: o includes +b_v after softmax-normalize (rows sum to 1),
    # so c1 = b_v @ w_proj + b_proj is a constant row added post-proj.
    c1 = b_v.astype(np.float64) @ np.asarray(w_proj, np.float64) + np.asarray(
        b_proj, np.float64
    )
    c1 = c1.astype(np.float32)
    bfc2 = np.asarray(b_fc2, np.float32)
    ln1_scale = np.asarray(ln1_scale, np.float32)
    ln1_bias = np.asarray(ln1_bias, np.float32)
    ln2_scale = np.asarray(ln2_scale, np.float32)
    ln2_bias = np.asarray(ln2_bias, np.float32)
    ln1_triv = bool(np.all(ln1_scale == 1) and np.all(ln1_bias == 0))
    ln2_triv = bool(np.all(ln2_scale == 1) and np.all(ln2_bias == 0))
    qk_triv = bool(np.all(b_qkv[: 2 * D] == 0))
    apply_c1 = bool(np.any(c1 != 0))
    apply_bfc2 = bool(np.any(bfc2 != 0))

    nc = _get_nc(ln1_triv, ln2_triv, qk_triv, apply_c1, apply_bfc2)

    base = {
        "w_qkv": np.asarray(w_qkv, np.float16),
        "w_proj": np.asarray(w_proj, np.float16),
        "w_fc1": np.asarray(w_fc1, np.float16),
        "w_fc2": np.asarray(w_fc2, np.float16),
        "b_qkv": b_qkv,
        "b_fc1": np.asarray(b_fc1, np.float32),
        "ln1_scale": ln1_scale,
        "ln1_bias": ln1_bias,
        "ln2_scale": ln2_scale,
        "ln2_bias": ln2_bias,
    }
    if apply_c1:
        base["c1"] = c1
    if apply_bfc2:
        base["b_fc2c"] = bfc2

    in_maps = [dict(base, x=np.ascontiguousarray(x[i])) for i in range(B)]
    last_err = None
    for _attempt in range(3):
        try:
            res = run_bass_kernel_spmd(nc, in_maps, core_ids=list(range(B)))
            break
        except Exception as e:  # transient NRT/axon worker failures recover on retry
            last_err = e
            import time as _time

            _time.sleep(2.0)
    else:
        raise last_err
    out = np.stack([res.results[i]["y"] for i in range(B)], axis=0)
    return np.ascontiguousarray(out.astype(np.float32))


# revision 28
# speedup vs baseline: 1.0261x; 1.0261x over previous
"""Trainium2 Bass kernel for a dense transformer block (pre-LN MHA + MLP).

Sharding: pure data parallel — batch (8) maps 1:1 onto the 8 NeuronCores.
Each core runs the full block on one [1024, 1024] slice with replicated
weights (host-cast to fp16 for the tensor engine; fp32 residual path).

Self-contained: hardcodes all shapes from the problem spec.
"""

from contextlib import ExitStack

import numpy as np

import concourse.bass as bass
import concourse.tile as tile
from concourse import bacc, mybir
from concourse.bass import ts
from concourse.bass_utils import run_bass_kernel_spmd
from concourse.masks import make_identity

F32 = mybir.dt.float32
F16 = mybir.dt.float16
AF = mybir.ActivationFunctionType
ALU = mybir.AluOpType

P = 128          # partitions
N = 1024         # tokens per core
D = 1024         # model dim
KC = D // P      # 8 contraction chunks of 128
HEADS = 16
HD = 64          # head dim
HID = 4096
EPS = 1e-6
NT = N // 512    # 2 chunks of 512 tokens
MT = N // P      # 8 token tiles of 128
SCALE = HD ** -0.5


def build_block(ln1_triv, ln2_triv, qk_triv, apply_c1, apply_bfc2):
    nc = bacc.Bacc("TRN2", target_bir_lowering=False, debug=False, num_devices=8)

    x_d = nc.dram_tensor("x", [N, D], F32, kind="ExternalInput")
    wqkv_d = nc.dram_tensor("w_qkv", [D, 3 * D], F16, kind="ExternalInput")
    wproj_d = nc.dram_tensor("w_proj", [D, D], F16, kind="ExternalInput")
    wfc1_d = nc.dram_tensor("w_fc1", [D, HID], F16, kind="ExternalInput")
    wfc2_d = nc.dram_tensor("w_fc2", [HID, D], F16, kind="ExternalInput")
    bqkv_d = nc.dram_tensor("b_qkv", [3 * D], F32, kind="ExternalInput")
    bfc1_d = nc.dram_tensor("b_fc1", [HID], F32, kind="ExternalInput")
    ln1s_d = nc.dram_tensor("ln1_scale", [D], F32, kind="ExternalInput")
    ln1b_d = nc.dram_tensor("ln1_bias", [D], F32, kind="ExternalInput")
    ln2s_d = nc.dram_tensor("ln2_scale", [D], F32, kind="ExternalInput")
    ln2b_d = nc.dram_tensor("ln2_bias", [D], F32, kind="ExternalInput")
    c1_d = nc.dram_tensor("c1", [D], F32, kind="ExternalInput") if apply_c1 else None
    bfc2_d = (
        nc.dram_tensor("b_fc2c", [D], F32, kind="ExternalInput") if apply_bfc2 else None
    )
    y_d = nc.dram_tensor("y", [N, D], F16, kind="ExternalOutput")

    # [(kc p), n] -> [p, kc, n] views for weight loads (lhsT layout)
    wqkv_v = wqkv_d.ap().rearrange("(kc p) n -> p kc n", p=P)
    wproj_v = wproj_d.ap().rearrange("(kc p) n -> p kc n", p=P)
    wfc1_v = wfc1_d.ap().rearrange("(kc p) n -> p kc n", p=P)
    wfc2_v = wfc2_d.ap().rearrange("(kc p) n -> p kc n", p=P)

    with tile.TileContext(nc) as tc, ExitStack() as ctx:
        ep = ctx.enter_context
        constp = ep(tc.tile_pool(name="const", bufs=1))
        xload = ep(tc.tile_pool(name="xload", bufs=2))
        x1p = ep(tc.tile_pool(name="x1", bufs=1))
        htmpp = ep(tc.tile_pool(name="htmp", bufs=2))
        hTp = ep(tc.tile_pool(name="hT", bufs=1))
        qTp = ep(tc.tile_pool(name="qT", bufs=1))
        kTp = ep(tc.tile_pool(name="kT", bufs=1))
        vp = ep(tc.tile_pool(name="vv", bufs=1))
        oTp = ep(tc.tile_pool(name="oT", bufs=1))
        probsp = ep(tc.tile_pool(name="probs", bufs=3))
        wp = ep(tc.tile_pool(name="w", bufs=4))
        statsp = ep(tc.tile_pool(name="stats", bufs=4))
        otmpp = ep(tc.tile_pool(name="otmp", bufs=1))
        pmp = ep(tc.tile_pool(name="pm", bufs=2, space="PSUM"))
        pvp = ep(tc.tile_pool(name="pv", bufs=1, space="PSUM"))
        ptp = ep(tc.tile_pool(name="pt", bufs=2, space="PSUM"))

        # ---- first x tile load goes out before anything else ----
        # two half-DMAs so bn_stats on the first half starts sooner
        x_t0 = xload.tile([P, D], F32, tag="x_t")
        nc.sync.dma_start(x_t0[:, 0:512], x_d.ap()[ts(0, P), 0:512])
        nc.sync.dma_start(x_t0[:, 512:1024], x_d.ap()[ts(0, P), 512:1024])

        # ---- constants (gpsimd queue; keeps sync queue on x) ----
        ident = constp.tile([P, P], F16)
        make_identity(nc, ident[:])
        eps_t = constp.tile([P, 1], F32)
        nc.vector.memset(eps_t[:], EPS)
        ones_t = constp.tile([P, HD], F16)
        nc.vector.memset(ones_t[:], 1.0)
        ln1s = constp.tile([P, KC], F32)
        nc.gpsimd.dma_start(ln1s[:], ln1s_d.ap().rearrange("(k p) -> p k", p=P))
        ln1b = constp.tile([P, KC], F32)
        nc.gpsimd.dma_start(ln1b[:], ln1b_d.ap().rearrange("(k p) -> p k", p=P))
        ln2s = constp.tile([P, KC], F32)
        nc.gpsimd.dma_start(ln2s[:], ln2s_d.ap().rearrange("(k p) -> p k", p=P))
        ln2b = constp.tile([P, KC], F32)
        nc.gpsimd.dma_start(ln2b[:], ln2b_d.ap().rearrange("(k p) -> p k", p=P))
        bqk = constp.tile([P, 16], F32)  # q,k bias columns (out_c 0..2047)
        bqkv_v = bqkv_d.ap().rearrange("(m p) -> p m", p=P)
        nc.gpsimd.dma_start(bqk[:], bqkv_v[:, 0:16])
        bfc1 = constp.tile([P, HID // P], F32)
        nc.gpsimd.dma_start(bfc1[:], bfc1_d.ap().rearrange("(m p) -> p m", p=P))
        if apply_c1:
            c1row = constp.tile([P, D], F32)
            src = c1_d.ap()
            nc.gpsimd.dma_start(
                c1row[:],
                bass.AP(tensor=src.tensor, offset=src.offset, ap=[[0, P], [1, D]]),
            )
        if apply_bfc2:
            b2row = constp.tile([P, D], F32)
            src = bfc2_d.ap()
            nc.gpsimd.dma_start(
                b2row[:],
                bass.AP(tensor=src.tensor, offset=src.offset, ap=[[0, P], [1, D]]),
            )

        hT = hTp.tile([P, KC, N], F16, tag="hT")

        def layer_norm_to_hT(src_ap, out_hT, s_cols, b_cols, mt, trivial):
            """LN over free dim of src [128, 1024]; write transposed fp16 into
            out_hT[:, kc, mt*128:...]. Work split across DVE/GPSIMD/ACT."""
            st = statsp.tile([P, 2, 6], F32)
            xr = src_ap.rearrange("p (a b) -> p a b", b=512)
            nc.vector.bn_stats(st[:, 0, :], xr[:, 0, :])
            nc.vector.bn_stats(st[:, 1, :], xr[:, 1, :])
            mv = statsp.tile([P, 2], F32)
            nc.vector.bn_aggr(mv[:], st[:])
            rstd = statsp.tile([P, 1], F32)
            nc.scalar.activation(rstd[:], mv[:, 1:2], AF.Sqrt, bias=eps_t[:])
            nc.vector.reciprocal(rstd[:], rstd[:])
            h = htmpp.tile([P, D], F16)
            nc.vector.tensor_scalar(
                out=h[:, 0:512], in0=src_ap[:, 0:512], scalar1=mv[:, 0:1],
                scalar2=rstd[:], op0=ALU.subtract, op1=ALU.mult,
            )
            nc.gpsimd.tensor_scalar(
                out=h[:, 512:1024], in0=src_ap[:, 512:1024], scalar1=mv[:, 0:1],
                scalar2=rstd[:], op0=ALU.subtract, op1=ALU.mult,
            )
            for kc in range(KC):
                pt_t = ptp.tile([P, P], F16, tag="pt")
                nc.tensor.transpose(pt_t[:], h[:, ts(kc, P)], ident[:])
                dst = out_hT[:, kc, ts(mt, P)]
                if trivial:
                    nc.scalar.copy(dst, pt_t[:])
                else:
                    nc.vector.tensor_scalar(
                        out=dst, in0=pt_t[:],
                        scalar1=s_cols[:, kc : kc + 1], scalar2=b_cols[:, kc : kc + 1],
                        op0=ALU.mult, op1=ALU.add,
                    )

        # ---- phase 1: LN1 + transpose ----
        for mt in range(MT):
            if mt == 0:
                x_t = x_t0
            else:
                x_t = xload.tile([P, D], F32, tag="x_t")
                nc.sync.dma_start(x_t[:, 0:512], x_d.ap()[ts(mt, P), 0:512])
                nc.sync.dma_start(x_t[:, 512:1024], x_d.ap()[ts(mt, P), 512:1024])
            layer_norm_to_hT(x_t[:], hT, ln1s, ln1b, mt, ln1_triv)

        # ---- phase 2: qkv (nt-outer so chains start once half of hT is up) --
        qT = qTp.tile([P, KC, N], F16, tag="qT")
        kT = kTp.tile([P, KC, N], F16, tag="kT")
        v_sb = vp.tile([P, MT, HEADS * (HD + 1)], F16, tag="vv")

        def wpiece(view, n0):
            t = wp.tile([P, KC, 512], F16, tag="w")
            nc.sync.dma_start(t[:], view[:, :, n0 : n0 + 512])
            return t

        for half in range(2):  # 0: q (cols 0:1024), 1: k (cols 1024:2048)
            pieces = [wpiece(wqkv_v, half * 1024), wpiece(wqkv_v, half * 1024 + 512)]
            dst_t = qT if half == 0 else kT
            for nt in range(NT):
                for mc_l in range(8):
                    mc = half * 8 + mc_l
                    piece = pieces[mc_l // 4]
                    ps = pmp.tile([P, 512], F32, tag="pm")
                    for kc in range(KC):
                        nc.tensor.matmul(
                            ps[:], piece[:, kc, ts(mc_l % 4, P)],
                            hT[:, kc, ts(nt, 512)],
                            start=(kc == 0), stop=(kc == KC - 1),
                        )
                    dst = dst_t[:, mc_l, ts(nt, 512)]
                    if qk_triv:
                        if mc_l % 2 == 0:
                            nc.vector.tensor_copy(dst, ps[:])
                        else:
                            nc.scalar.copy(dst, ps[:])
                    else:
                        nc.vector.tensor_scalar(
                            out=dst, in0=ps[:], scalar1=bqk[:, mc : mc + 1],
                            scalar2=None, op0=ALU.add,
                        )

        # ---- attention emission helpers ----
        def scores_pair(h, probs, mk):
            """one [128,1024] scoresT stripe + exp into probs[:, mk, :]."""
            mc_h = h // 2
            pr = (h % 2) * HD
            ps = pmp.tile([P, N], F32, tag="pm")
            for nq in range(NT):
                nc.tensor.matmul(
                    ps[:, ts(nq, 512)],
                    kT[pr : pr + HD, mc_h, ts(mk, P)],
                    qT[pr : pr + HD, mc_h, ts(nq, 512)],
                    start=True, stop=True,
                )
            nc.scalar.activation(probs[:, mk, :], ps[:], AF.Exp, scale=SCALE)

        def scores_group(h):
            probs = probsp.tile([P, KC, N], F16, tag="probs")
            for mk in range(MT):
                scores_pair(h, probs, mk)
            return probs

        oT = oTp.tile([P, KC, N], F16, tag="oT")

        # v (token-major, ones column per head at stride 65), interleaved with
        # the first two heads' score stripes so their exps overlap v matmuls
        v_pieces = [wpiece(wqkv_v, n0) for n0 in (2048, 2560)]
        # odd heads first: even heads (base partition 0) can write oT without
        # a shifting DMA, so the last-processed heads retire fastest
        HORD = [h for h in range(HEADS) if h % 2] + [h for h in range(HEADS) if not h % 2]
        probs_n_alloc = [0]

        def probs_tile():
            # every 4th tile borrows the hT slot (hT is dead once v is built;
            # the first borrowed tile is only written deep into attention) —
            # an effective 4-deep probs rotation
            i = probs_n_alloc[0]
            probs_n_alloc[0] += 1
            if i % 4 == 3:
                pb = hTp.tile([P, KC, N], F16, tag="hT")
            else:
                pb = probsp.tile([P, KC, N], F16, tag="probs")
            return pb

        probs_q = [probs_tile(), probs_tile()]
        for mt in range(MT):
            v_row = v_sb[:, mt, :].rearrange("p (h c) -> p h c", c=HD + 1)
            nc.vector.memset(v_row[:, :, HD : HD + 1], 1.0)
            ps = pmp.tile([P, N], F32, tag="pm")
            for nv in range(2):
                for kc in range(KC):
                    nc.tensor.matmul(
                        ps[:, ts(nv, 512)], hT[:, kc, ts(mt, P)],
                        v_pieces[nv][:, kc, :],
                        start=(kc == 0), stop=(kc == KC - 1),
                    )
            dst = v_row[:, :, 0:HD]
            src = ps[:].rearrange("p (h c) -> p h c", c=HD)
            if mt % 2 == 0:
                nc.vector.tensor_copy(dst, src)
            else:
                nc.scalar.copy(dst, src)
            # first two heads' score stripes ride the (here idle) pt pool so
            # the v chains keep both pm slots
            for hh in range(2):
                h0 = HORD[hh]
                mc0 = h0 // 2
                pr0 = (h0 % 2) * HD
                for nq in range(NT):
                    sps = ptp.tile([P, 512], F32, tag="pt")
                    nc.tensor.matmul(
                        sps[:],
                        kT[pr0 : pr0 + HD, mc0, ts(mt, P)],
                        qT[pr0 : pr0 + HD, mc0, ts(nq, 512)],
                        start=True, stop=True,
                    )
                    nc.scalar.activation(
                        probs_q[hh][:, mt, ts(nq, 512)], sps[:], AF.Exp, scale=SCALE
                    )

        # w_proj load early (streams behind attention)
        proj_pieces = [wpiece(wproj_v, n0) for n0 in (0, 512)]

        # ---- phase 3: attention main loop ----
        for hi, h in enumerate(HORD):
            probs_h = probs_q.pop(0)
            if hi + 2 < HEADS:
                probs_next = probs_tile()
                probs_q.append(probs_next)
                todo = list(range(MT))
                h_next = HORD[hi + 2]
            else:
                probs_next, todo, h_next = None, [], None
            mc_h = h // 2
            pr = (h % 2) * HD
            pav = pvp.tile([P, N], F32, tag="pv")
            for j, (nq, mk) in enumerate([(a, b) for a in range(NT) for b in range(MT)]):
                nc.tensor.matmul(
                    pav[0 : HD + 1, ts(nq, 512)],
                    v_sb[:, mk, h * (HD + 1) : (h + 1) * (HD + 1)],
                    probs_h[:, mk, ts(nq, 512)],
                    start=(mk == 0), stop=(mk == MT - 1),
                    skip_group_check=True,
                )
                if j % 2 == 1 and j < 12 and todo:
                    scores_pair(h_next, probs_next, todo.pop(0))
            # last two score stripes land here so PE stays busy while the
            # drain copy below holds the single pv slot
            while todo:
                scores_pair(h_next, probs_next, todo.pop(0))
            # drain psum promptly (frees the slot for the next head)
            av_f = otmpp.tile([HD + 1, N], F32, tag="av_f")
            nc.vector.tensor_copy(av_f[:], pav[0 : HD + 1, :])
            nc.vector.reciprocal(av_f[HD : HD + 1, :], av_f[HD : HD + 1, :])
            srow16 = otmpp.tile([1, N], F16, tag="srow16")
            nc.vector.tensor_copy(srow16[:], av_f[HD : HD + 1, :])
            if pr == 0:
                for nq in range(NT):
                    rb = ptp.tile([HD, 512], F32, tag="pt")
                    nc.tensor.matmul(
                        rb[:], ones_t[0:1, 0:HD], srow16[:, ts(nq, 512)],
                        start=True, stop=True,
                    )
                    nc.vector.tensor_mul(
                        oT[0:HD, mc_h, ts(nq, 512)], av_f[0:HD, ts(nq, 512)], rb[:]
                    )
            else:
                o_t = otmpp.tile([HD, N], F16, tag="o_t")
                for nq in range(NT):
                    rb = ptp.tile([HD, 512], F32, tag="pt")
                    nc.tensor.matmul(
                        rb[:], ones_t[0:1, 0:HD], srow16[:, ts(nq, 512)],
                        start=True, stop=True,
                    )
                    nc.vector.tensor_mul(
                        o_t[:, ts(nq, 512)], av_f[0:HD, ts(nq, 512)], rb[:]
                    )
                nc.sync.dma_start(oT[pr : pr + HD, mc_h, :], o_t[:])

        # ---- phase 4+5: proj + residual -> x1, LN2 fused per tile ----
        # (fused so LN2's DVE/ACT chains pipeline behind each proj epilogue
        # instead of queueing after all of proj on the in-order engines)
        x1 = x1p.tile([P, MT, D], F16)
        h2T = hTp.tile([P, KC, N], F16, tag="hT")
        for mt in range(MT):
            x_t = xload.tile([P, D], F32, tag="x_t")
            nc.sync.dma_start(x_t[:], x_d.ap()[ts(mt, P), :])
            ps = pmp.tile([P, N], F32, tag="pm")
            for np_ in range(NT):
                for kc in range(KC):
                    nc.tensor.matmul(
                        ps[:, ts(np_, 512)], oT[:, kc, ts(mt, P)],
                        proj_pieces[np_][:, kc, :],
                        start=(kc == 0), stop=(kc == KC - 1),
                    )
            nc.vector.tensor_add(x1[:, mt, :], ps[:], x_t[:])
            if apply_c1:
                nc.vector.tensor_add(x1[:, mt, :], x1[:, mt, :], c1row[:])
            layer_norm_to_hT(x1[:, mt, :], h2T, ln2s, ln2b, mt, ln2_triv)

        # ---- phase 6: fc1 (gelu) ----
        # a1T groups g=0..3 each [128, 8, 1024] fp16, reusing attention pools
        a1_pools = [(qTp, "qT"), (kTp, "kT"), (vp, "vv"), (oTp, "oT")]
        a1 = []
        for pool, tag in a1_pools:
            a1_g = pool.tile([P, KC, N], F16, tag=tag)
            a1.append(a1_g)
        for p8 in range(8):  # 512-wide hidden column pieces
            w1_t = wpiece(wfc1_v, p8 * 512)
            for nt in range(NT):
                for mh_l in range(4):
                    mhg = p8 * 4 + mh_l
                    ps = pmp.tile([P, 512], F32, tag="pm")
                    for kc in range(KC):
                        nc.tensor.matmul(
                            ps[:], w1_t[:, kc, ts(mh_l, P)], h2T[:, kc, ts(nt, 512)],
                            start=(kc == 0), stop=(kc == KC - 1),
                        )
                    nc.scalar.activation(
                        a1[mhg // 8][:, mhg % 8, ts(nt, 512)], ps[:],
                        AF.Gelu_apprx_tanh, bias=bfc1[:, mhg : mhg + 1],
                    )

        # ---- phase 7: fc2 + residual -> y ----
        # stream w2 in half-group pieces; accumulate partials into x1 per group
        for g in range(4):
            wa = wp.tile([P, 4, N], F16, tag="w")
            nc.sync.dma_start(wa[:], wfc2_v[:, g * 8 : g * 8 + 4, :])
            wb = wp.tile([P, 4, N], F16, tag="w")
            nc.sync.dma_start(wb[:], wfc2_v[:, g * 8 + 4 : g * 8 + 8, :])
            for mt in range(MT):
                ps = pmp.tile([P, N], F32, tag="pm")
                for ncol in range(NT):
                    for kc in range(KC):
                        w2 = wa if kc < 4 else wb
                        nc.tensor.matmul(
                            ps[:, ts(ncol, 512)], a1[g][:, kc, ts(mt, P)],
                            w2[:, kc % 4, ts(ncol, 512)],
                            start=(kc == 0), stop=(kc == KC - 1),
                        )
                nc.vector.tensor_add(x1[:, mt, :], ps[:], x1[:, mt, :])
        for mt in range(MT):
            if apply_bfc2:
                nc.vector.tensor_add(x1[:, mt, :], x1[:, mt, :], b2row[:])
            nc.sync.dma_start(y_d.ap()[ts(mt, P), :], x1[:, mt, :])

    nc.compile()
    return nc


_cache = {}


def _get_nc(*key):
    if key not in _cache:
        _cache[key] = build_block(*key)
    return _cache[key]


def kernel(
    x, w_qkv, b_qkv, w_proj, b_proj, ln1_scale, ln1_bias,
    ln2_scale, ln2_bias, w_fc1, b_fc1, w_fc2, b_fc2,
):
    x = np.asarray(x, np.float32)
    B = x.shape[0]
    b_qkv = np.asarray(b_qkv, np.float32)
    b_v = b_qkv[2 * D :]
    # exact folds: o includes +b_v after softmax-normalize (rows sum to 1),
    # so c1 = b_v @ w_proj + b_proj is a constant row added post-proj.
    c1 = b_v.astype(np.float64) @ np.asarray(w_proj, np.float64) + np.asarray(
        b_proj, np.float64
    )
    c1 = c1.astype(np.float32)
    bfc2 = np.asarray(b_fc2, np.float32)
    ln1_scale = np.asarray(ln1_scale, np.float32)
    ln1_bias = np.asarray(ln1_bias, np.float32)
    ln2_scale = np.asarray(ln2_scale, np.float32)
    ln2_bias = np.asarray(ln2_bias, np.float32)
    ln1_triv = bool(np.all(ln1_scale == 1) and np.all(ln1_bias == 0))
    ln2_triv = bool(np.all(ln2_scale == 1) and np.all(ln2_bias == 0))
    qk_triv = bool(np.all(b_qkv[: 2 * D] == 0))
    apply_c1 = bool(np.any(c1 != 0))
    apply_bfc2 = bool(np.any(bfc2 != 0))

    nc = _get_nc(ln1_triv, ln2_triv, qk_triv, apply_c1, apply_bfc2)

    base = {
        "w_qkv": np.asarray(w_qkv, np.float16),
        "w_proj": np.asarray(w_proj, np.float16),
        "w_fc1": np.asarray(w_fc1, np.float16),
        "w_fc2": np.asarray(w_fc2, np.float16),
        "b_qkv": b_qkv,
        "b_fc1": np.asarray(b_fc1, np.float32),
        "ln1_scale": ln1_scale,
        "ln1_bias": ln1_bias,
        "ln2_scale": ln2_scale,
        "ln2_bias": ln2_bias,
    }
    if apply_c1:
        base["c1"] = c1
    if apply_bfc2:
        base["b_fc2c"] = bfc2

    in_maps = [dict(base, x=np.ascontiguousarray(x[i])) for i in range(B)]
    last_err = None
    for _attempt in range(3):
        try:
            res = run_bass_kernel_spmd(nc, in_maps, core_ids=list(range(B)))
            break
        except Exception as e:  # transient NRT/axon worker failures recover on retry
            last_err = e
            import time as _time

            _time.sleep(2.0)
    else:
        raise last_err
    out = np.stack([res.results[i]["y"] for i in range(B)], axis=0)
    return np.ascontiguousarray(out.astype(np.float32))


# revision 29
# speedup vs baseline: 1.0268x; 1.0007x over previous
"""Trainium2 Bass kernel for a dense transformer block (pre-LN MHA + MLP).

Sharding: pure data parallel — batch (8) maps 1:1 onto the 8 NeuronCores.
Each core runs the full block on one [1024, 1024] slice with replicated
weights (host-cast to fp16 for the tensor engine; fp32 residual path).

Self-contained: hardcodes all shapes from the problem spec.
"""

from contextlib import ExitStack

import numpy as np

import concourse.bass as bass
import concourse.tile as tile
from concourse import bacc, mybir
from concourse.bass import ts
from concourse.bass_utils import run_bass_kernel_spmd
from concourse.masks import make_identity

F32 = mybir.dt.float32
F16 = mybir.dt.float16
AF = mybir.ActivationFunctionType
ALU = mybir.AluOpType

P = 128          # partitions
N = 1024         # tokens per core
D = 1024         # model dim
KC = D // P      # 8 contraction chunks of 128
HEADS = 16
HD = 64          # head dim
HID = 4096
EPS = 1e-6
NT = N // 512    # 2 chunks of 512 tokens
MT = N // P      # 8 token tiles of 128
SCALE = HD ** -0.5


def build_block(ln1_triv, ln2_triv, qk_triv, apply_c1, apply_bfc2):
    nc = bacc.Bacc("TRN2", target_bir_lowering=False, debug=False, num_devices=8)

    x_d = nc.dram_tensor("x", [N, D], F32, kind="ExternalInput")
    wqkv_d = nc.dram_tensor("w_qkv", [D, 3 * D], F16, kind="ExternalInput")
    wproj_d = nc.dram_tensor("w_proj", [D, D], F16, kind="ExternalInput")
    wfc1_d = nc.dram_tensor("w_fc1", [D, HID], F16, kind="ExternalInput")
    wfc2_d = nc.dram_tensor("w_fc2", [HID, D], F16, kind="ExternalInput")
    bqkv_d = nc.dram_tensor("b_qkv", [3 * D], F32, kind="ExternalInput")
    bfc1_d = nc.dram_tensor("b_fc1", [HID], F32, kind="ExternalInput")
    ln1s_d = nc.dram_tensor("ln1_scale", [D], F32, kind="ExternalInput")
    ln1b_d = nc.dram_tensor("ln1_bias", [D], F32, kind="ExternalInput")
    ln2s_d = nc.dram_tensor("ln2_scale", [D], F32, kind="ExternalInput")
    ln2b_d = nc.dram_tensor("ln2_bias", [D], F32, kind="ExternalInput")
    c1_d = nc.dram_tensor("c1", [D], F32, kind="ExternalInput") if apply_c1 else None
    bfc2_d = (
        nc.dram_tensor("b_fc2c", [D], F32, kind="ExternalInput") if apply_bfc2 else None
    )
    y_d = nc.dram_tensor("y", [N, D], F16, kind="ExternalOutput")

    # [(kc p), n] -> [p, kc, n] views for weight loads (lhsT layout)
    wqkv_v = wqkv_d.ap().rearrange("(kc p) n -> p kc n", p=P)
    wproj_v = wproj_d.ap().rearrange("(kc p) n -> p kc n", p=P)
    wfc1_v = wfc1_d.ap().rearrange("(kc p) n -> p kc n", p=P)
    wfc2_v = wfc2_d.ap().rearrange("(kc p) n -> p kc n", p=P)

    with tile.TileContext(nc) as tc, ExitStack() as ctx:
        ep = ctx.enter_context
        constp = ep(tc.tile_pool(name="const", bufs=1))
        xload = ep(tc.tile_pool(name="xload", bufs=2))
        x1p = ep(tc.tile_pool(name="x1", bufs=1))
        htmpp = ep(tc.tile_pool(name="htmp", bufs=2))
        hTp = ep(tc.tile_pool(name="hT", bufs=1))
        qTp = ep(tc.tile_pool(name="qT", bufs=1))
        kTp = ep(tc.tile_pool(name="kT", bufs=1))
        vp = ep(tc.tile_pool(name="vv", bufs=1))
        oTp = ep(tc.tile_pool(name="oT", bufs=1))
        probsp = ep(tc.tile_pool(name="probs", bufs=3))
        wp = ep(tc.tile_pool(name="w", bufs=4))
        statsp = ep(tc.tile_pool(name="stats", bufs=4))
        otmpp = ep(tc.tile_pool(name="otmp", bufs=1))
        pmp = ep(tc.tile_pool(name="pm", bufs=2, space="PSUM"))
        pvp = ep(tc.tile_pool(name="pv", bufs=1, space="PSUM"))
        ptp = ep(tc.tile_pool(name="pt", bufs=2, space="PSUM"))

        # ---- first x tile load goes out before anything else ----
        # two half-DMAs so bn_stats on the first half starts sooner
        x_t0 = xload.tile([P, D], F32, tag="x_t")
        nc.sync.dma_start(x_t0[:, 0:512], x_d.ap()[ts(0, P), 0:512])
        nc.sync.dma_start(x_t0[:, 512:1024], x_d.ap()[ts(0, P), 512:1024])

        # ---- constants (gpsimd queue; keeps sync queue on x) ----
        ident = constp.tile([P, P], F16)
        make_identity(nc, ident[:])
        eps_t = constp.tile([P, 1], F32)
        nc.vector.memset(eps_t[:], EPS)
        ones_t = constp.tile([P, HD], F16)
        nc.vector.memset(ones_t[:], 1.0)
        ln1s = constp.tile([P, KC], F32)
        nc.gpsimd.dma_start(ln1s[:], ln1s_d.ap().rearrange("(k p) -> p k", p=P))
        ln1b = constp.tile([P, KC], F32)
        nc.gpsimd.dma_start(ln1b[:], ln1b_d.ap().rearrange("(k p) -> p k", p=P))
        ln2s = constp.tile([P, KC], F32)
        nc.gpsimd.dma_start(ln2s[:], ln2s_d.ap().rearrange("(k p) -> p k", p=P))
        ln2b = constp.tile([P, KC], F32)
        nc.gpsimd.dma_start(ln2b[:], ln2b_d.ap().rearrange("(k p) -> p k", p=P))
        bqk = constp.tile([P, 16], F32)  # q,k bias columns (out_c 0..2047)
        bqkv_v = bqkv_d.ap().rearrange("(m p) -> p m", p=P)
        nc.gpsimd.dma_start(bqk[:], bqkv_v[:, 0:16])
        bfc1 = constp.tile([P, HID // P], F32)
        nc.gpsimd.dma_start(bfc1[:], bfc1_d.ap().rearrange("(m p) -> p m", p=P))
        if apply_c1:
            c1row = constp.tile([P, D], F32)
            src = c1_d.ap()
            nc.gpsimd.dma_start(
                c1row[:],
                bass.AP(tensor=src.tensor, offset=src.offset, ap=[[0, P], [1, D]]),
            )
        if apply_bfc2:
            b2row = constp.tile([P, D], F32)
            src = bfc2_d.ap()
            nc.gpsimd.dma_start(
                b2row[:],
                bass.AP(tensor=src.tensor, offset=src.offset, ap=[[0, P], [1, D]]),
            )

        hT = hTp.tile([P, KC, N], F16, tag="hT")

        def layer_norm_to_hT(src_ap, out_hT, s_cols, b_cols, mt, trivial):
            """LN over free dim of src [128, 1024]; write transposed fp16 into
            out_hT[:, kc, mt*128:...]. Work split across DVE/GPSIMD/ACT."""
            st = statsp.tile([P, 2, 6], F32)
            xr = src_ap.rearrange("p (a b) -> p a b", b=512)
            nc.vector.bn_stats(st[:, 0, :], xr[:, 0, :])
            nc.vector.bn_stats(st[:, 1, :], xr[:, 1, :])
            mv = statsp.tile([P, 2], F32)
            nc.vector.bn_aggr(mv[:], st[:])
            rstd = statsp.tile([P, 1], F32)
            nc.scalar.activation(rstd[:], mv[:, 1:2], AF.Sqrt, bias=eps_t[:])
            nc.vector.reciprocal(rstd[:], rstd[:])
            h = htmpp.tile([P, D], F16)
            nc.vector.tensor_scalar(
                out=h[:, 0:512], in0=src_ap[:, 0:512], scalar1=mv[:, 0:1],
                scalar2=rstd[:], op0=ALU.subtract, op1=ALU.mult,
            )
            nc.gpsimd.tensor_scalar(
                out=h[:, 512:1024], in0=src_ap[:, 512:1024], scalar1=mv[:, 0:1],
                scalar2=rstd[:], op0=ALU.subtract, op1=ALU.mult,
            )
            for kc in range(KC):
                pt_t = ptp.tile([P, P], F16, tag="pt")
                nc.tensor.transpose(pt_t[:], h[:, ts(kc, P)], ident[:])
                dst = out_hT[:, kc, ts(mt, P)]
                if trivial:
                    nc.scalar.copy(dst, pt_t[:])
                else:
                    nc.vector.tensor_scalar(
                        out=dst, in0=pt_t[:],
                        scalar1=s_cols[:, kc : kc + 1], scalar2=b_cols[:, kc : kc + 1],
                        op0=ALU.mult, op1=ALU.add,
                    )

        # ---- phase 1: LN1 + transpose ----
        for mt in range(MT):
            if mt == 0:
                x_t = x_t0
            else:
                x_t = xload.tile([P, D], F32, tag="x_t")
                nc.sync.dma_start(x_t[:, 0:512], x_d.ap()[ts(mt, P), 0:512])
                nc.sync.dma_start(x_t[:, 512:1024], x_d.ap()[ts(mt, P), 512:1024])
            layer_norm_to_hT(x_t[:], hT, ln1s, ln1b, mt, ln1_triv)

        # ---- phase 2: qkv (nt-outer so chains start once half of hT is up) --
        qT = qTp.tile([P, KC, N], F16, tag="qT")
        kT = kTp.tile([P, KC, N], F16, tag="kT")
        v_sb = vp.tile([P, MT, HEADS * (HD + 1)], F16, tag="vv")

        def wpiece(view, n0):
            t = wp.tile([P, KC, 512], F16, tag="w")
            nc.sync.dma_start(t[:], view[:, :, n0 : n0 + 512])
            return t

        for half in range(2):  # 0: q (cols 0:1024), 1: k (cols 1024:2048)
            pieces = [wpiece(wqkv_v, half * 1024), wpiece(wqkv_v, half * 1024 + 512)]
            dst_t = qT if half == 0 else kT
            for nt in range(NT):
                for mc_l in range(8):
                    mc = half * 8 + mc_l
                    piece = pieces[mc_l // 4]
                    ps = pmp.tile([P, 512], F32, tag="pm")
                    for kc in range(KC):
                        nc.tensor.matmul(
                            ps[:], piece[:, kc, ts(mc_l % 4, P)],
                            hT[:, kc, ts(nt, 512)],
                            start=(kc == 0), stop=(kc == KC - 1),
                        )
                    dst = dst_t[:, mc_l, ts(nt, 512)]
                    if qk_triv:
                        if mc_l % 2 == 0:
                            nc.vector.tensor_copy(dst, ps[:])
                        else:
                            nc.scalar.copy(dst, ps[:])
                    else:
                        nc.vector.tensor_scalar(
                            out=dst, in0=ps[:], scalar1=bqk[:, mc : mc + 1],
                            scalar2=None, op0=ALU.add,
                        )

        # ---- attention emission helpers ----
        def scores_pair(h, probs, mk):
            """one [128,1024] scoresT stripe + exp into probs[:, mk, :]."""
            mc_h = h // 2
            pr = (h % 2) * HD
            ps = pmp.tile([P, N], F32, tag="pm")
            for nq in range(NT):
                nc.tensor.matmul(
                    ps[:, ts(nq, 512)],
                    kT[pr : pr + HD, mc_h, ts(mk, P)],
                    qT[pr : pr + HD, mc_h, ts(nq, 512)],
                    start=True, stop=True,
                )
            nc.scalar.activation(probs[:, mk, :], ps[:], AF.Exp, scale=SCALE)

        def scores_group(h):
            probs = probsp.tile([P, KC, N], F16, tag="probs")
            for mk in range(MT):
                scores_pair(h, probs, mk)
            return probs

        oT = oTp.tile([P, KC, N], F16, tag="oT")

        # v (token-major, ones column per head at stride 65), interleaved with
        # the first two heads' score stripes so their exps overlap v matmuls
        v_pieces = [wpiece(wqkv_v, n0) for n0 in (2048, 2560)]
        # odd heads first: even heads (base partition 0) can write oT without
        # a shifting DMA, so the last-processed heads retire fastest
        HORD = [h for h in range(HEADS) if h % 2] + [h for h in range(HEADS) if not h % 2]
        probs_n_alloc = [0]

        def probs_tile():
            # every 4th tile borrows the hT slot (hT is dead once v is built;
            # the first borrowed tile is only written deep into attention) —
            # an effective 4-deep probs rotation
            i = probs_n_alloc[0]
            probs_n_alloc[0] += 1
            if i % 4 == 3:
                pb = hTp.tile([P, KC, N], F16, tag="hT")
            else:
                pb = probsp.tile([P, KC, N], F16, tag="probs")
            return pb

        probs_q = [probs_tile(), probs_tile()]
        for mt in range(MT):
            v_row = v_sb[:, mt, :].rearrange("p (h c) -> p h c", c=HD + 1)
            nc.vector.memset(v_row[:, :, HD : HD + 1], 1.0)
            ps = pmp.tile([P, N], F32, tag="pm")
            for nv in range(2):
                for kc in range(KC):
                    nc.tensor.matmul(
                        ps[:, ts(nv, 512)], hT[:, kc, ts(mt, P)],
                        v_pieces[nv][:, kc, :],
                        start=(kc == 0), stop=(kc == KC - 1),
                    )
            dst = v_row[:, :, 0:HD]
            src = ps[:].rearrange("p (h c) -> p h c", c=HD)
            if mt % 2 == 0:
                nc.vector.tensor_copy(dst, src)
            else:
                nc.scalar.copy(dst, src)
            # first two heads' score stripes ride the (here idle) pt pool so
            # the v chains keep both pm slots
            for hh in range(2):
                h0 = HORD[hh]
                mc0 = h0 // 2
                pr0 = (h0 % 2) * HD
                for nq in range(NT):
                    sps = ptp.tile([P, 512], F32, tag="pt")
                    nc.tensor.matmul(
                        sps[:],
                        kT[pr0 : pr0 + HD, mc0, ts(mt, P)],
                        qT[pr0 : pr0 + HD, mc0, ts(nq, 512)],
                        start=True, stop=True,
                    )
                    nc.scalar.activation(
                        probs_q[hh][:, mt, ts(nq, 512)], sps[:], AF.Exp, scale=SCALE
                    )

        # w_proj load early (streams behind attention)
        proj_pieces = [wpiece(wproj_v, n0) for n0 in (0, 512)]

        # ---- phase 3: attention main loop ----
        for hi, h in enumerate(HORD):
            probs_h = probs_q.pop(0)
            if hi + 2 < HEADS:
                probs_next = probs_tile()
                probs_q.append(probs_next)
                todo = list(range(MT))
                h_next = HORD[hi + 2]
            else:
                probs_next, todo, h_next = None, [], None
            mc_h = h // 2
            pr = (h % 2) * HD
            pav = pvp.tile([P, N], F32, tag="pv")
            for j, (nq, mk) in enumerate([(a, b) for a in range(NT) for b in range(MT)]):
                nc.tensor.matmul(
                    pav[0 : HD + 1, ts(nq, 512)],
                    v_sb[:, mk, h * (HD + 1) : (h + 1) * (HD + 1)],
                    probs_h[:, mk, ts(nq, 512)],
                    start=(mk == 0), stop=(mk == MT - 1),
                    skip_group_check=True,
                )
                if j % 2 == 1 and j < 12 and todo:
                    scores_pair(h_next, probs_next, todo.pop(0))
            # last two score stripes land here so PE stays busy while the
            # drain copy below holds the single pv slot
            while todo:
                scores_pair(h_next, probs_next, todo.pop(0))
            # drain psum promptly (frees the slot for the next head)
            av_f = otmpp.tile([HD + 1, N], F32, tag="av_f")
            srow16 = otmpp.tile([1, N], F16, tag="srow16")
            last = hi == HEADS - 1
            if last:
                # final head: per-half chain so proj's kc7 can start sooner
                for nq in range(NT):
                    sl = ts(nq, 512)
                    nc.vector.tensor_copy(av_f[:, sl], pav[0 : HD + 1, sl])
                    nc.vector.reciprocal(av_f[HD : HD + 1, sl], av_f[HD : HD + 1, sl])
                    nc.vector.tensor_copy(srow16[:, sl], av_f[HD : HD + 1, sl])
                    rb = ptp.tile([HD, 512], F32, tag="pt")
                    nc.tensor.matmul(
                        rb[:], ones_t[0:1, 0:HD], srow16[:, sl],
                        start=True, stop=True,
                    )
                    nc.vector.tensor_mul(
                        oT[0:HD, mc_h, sl], av_f[0:HD, sl], rb[:]
                    )
            else:
                nc.vector.tensor_copy(av_f[:], pav[0 : HD + 1, :])
                nc.vector.reciprocal(av_f[HD : HD + 1, :], av_f[HD : HD + 1, :])
                nc.vector.tensor_copy(srow16[:], av_f[HD : HD + 1, :])
            if not last and pr == 0:
                for nq in range(NT):
                    rb = ptp.tile([HD, 512], F32, tag="pt")
                    nc.tensor.matmul(
                        rb[:], ones_t[0:1, 0:HD], srow16[:, ts(nq, 512)],
                        start=True, stop=True,
                    )
                    nc.vector.tensor_mul(
                        oT[0:HD, mc_h, ts(nq, 512)], av_f[0:HD, ts(nq, 512)], rb[:]
                    )
            elif pr != 0:
                o_t = otmpp.tile([HD, N], F16, tag="o_t")
                for nq in range(NT):
                    rb = ptp.tile([HD, 512], F32, tag="pt")
                    nc.tensor.matmul(
                        rb[:], ones_t[0:1, 0:HD], srow16[:, ts(nq, 512)],
                        start=True, stop=True,
                    )
                    nc.vector.tensor_mul(
                        o_t[:, ts(nq, 512)], av_f[0:HD, ts(nq, 512)], rb[:]
                    )
                nc.sync.dma_start(oT[pr : pr + HD, mc_h, :], o_t[:])

        # ---- phase 4+5: proj + residual -> x1, LN2 fused per tile ----
        # (fused so LN2's DVE/ACT chains pipeline behind each proj epilogue
        # instead of queueing after all of proj on the in-order engines)
        x1 = x1p.tile([P, MT, D], F16)
        h2T = hTp.tile([P, KC, N], F16, tag="hT")
        for mt in range(MT):
            x_t = xload.tile([P, D], F32, tag="x_t")
            nc.sync.dma_start(x_t[:], x_d.ap()[ts(mt, P), :])
            ps = pmp.tile([P, N], F32, tag="pm")
            for np_ in range(NT):
                for kc in range(KC):
                    nc.tensor.matmul(
                        ps[:, ts(np_, 512)], oT[:, kc, ts(mt, P)],
                        proj_pieces[np_][:, kc, :],
                        start=(kc == 0), stop=(kc == KC - 1),
                    )
            nc.vector.tensor_add(x1[:, mt, :], ps[:], x_t[:])
            if apply_c1:
                nc.vector.tensor_add(x1[:, mt, :], x1[:, mt, :], c1row[:])
            layer_norm_to_hT(x1[:, mt, :], h2T, ln2s, ln2b, mt, ln2_triv)

        # ---- phase 6: fc1 (gelu) ----
        # a1T groups g=0..3 each [128, 8, 1024] fp16, reusing attention pools
        a1_pools = [(qTp, "qT"), (kTp, "kT"), (vp, "vv"), (oTp, "oT")]
        a1 = []
        for pool, tag in a1_pools:
            a1_g = pool.tile([P, KC, N], F16, tag=tag)
            a1.append(a1_g)
        for p8 in range(8):  # 512-wide hidden column pieces
            w1_t = wpiece(wfc1_v, p8 * 512)
            for nt in range(NT):
                for mh_l in range(4):
                    mhg = p8 * 4 + mh_l
                    ps = pmp.tile([P, 512], F32, tag="pm")
                    for kc in range(KC):
                        nc.tensor.matmul(
                            ps[:], w1_t[:, kc, ts(mh_l, P)], h2T[:, kc, ts(nt, 512)],
                            start=(kc == 0), stop=(kc == KC - 1),
                        )
                    nc.scalar.activation(
                        a1[mhg // 8][:, mhg % 8, ts(nt, 512)], ps[:],
                        AF.Gelu_apprx_tanh, bias=bfc1[:, mhg : mhg + 1],
                    )

        # ---- phase 7: fc2 + residual -> y ----
        # stream w2 in half-group pieces; accumulate partials into x1 per group
        for g in range(4):
            wa = wp.tile([P, 4, N], F16, tag="w")
            nc.sync.dma_start(wa[:], wfc2_v[:, g * 8 : g * 8 + 4, :])
            wb = wp.tile([P, 4, N], F16, tag="w")
            nc.sync.dma_start(wb[:], wfc2_v[:, g * 8 + 4 : g * 8 + 8, :])
            for mt in range(MT):
                ps = pmp.tile([P, N], F32, tag="pm")
                for ncol in range(NT):
                    for kc in range(KC):
                        w2 = wa if kc < 4 else wb
                        nc.tensor.matmul(
                            ps[:, ts(ncol, 512)], a1[g][:, kc, ts(mt, P)],
                            w2[:, kc % 4, ts(ncol, 512)],
                            start=(kc == 0), stop=(kc == KC - 1),
                        )
                nc.vector.tensor_add(x1[:, mt, :], ps[:], x1[:, mt, :])
        for mt in range(MT):
            if apply_bfc2:
                nc.vector.tensor_add(x1[:, mt, :], x1[:, mt, :], b2row[:])
            nc.sync.dma_start(y_d.ap()[ts(mt, P), :], x1[:, mt, :])

    nc.compile()
    return nc


_cache = {}


def _get_nc(*key):
    if key not in _cache:
        _cache[key] = build_block(*key)
    return _cache[key]


def kernel(
    x, w_qkv, b_qkv, w_proj, b_proj, ln1_scale, ln1_bias,
    ln2_scale, ln2_bias, w_fc1, b_fc1, w_fc2, b_fc2,
):
    x = np.asarray(x, np.float32)
    B = x.shape[0]
    b_qkv = np.asarray(b_qkv, np.float32)
    b_v = b_qkv[2 * D :]
    # exact folds: o includes +b_v after softmax-normalize (rows sum to 1),
    # so c1 = b_v @ w_proj + b_proj is a constant row added post-proj.
    c1 = b_v.astype(np.float64) @ np.asarray(w_proj, np.float64) + np.asarray(
        b_proj, np.float64
    )
    c1 = c1.astype(np.float32)
    bfc2 = np.asarray(b_fc2, np.float32)
    ln1_scale = np.asarray(ln1_scale, np.float32)
    ln1_bias = np.asarray(ln1_bias, np.float32)
    ln2_scale = np.asarray(ln2_scale, np.float32)
    ln2_bias = np.asarray(ln2_bias, np.float32)
    ln1_triv = bool(np.all(ln1_scale == 1) and np.all(ln1_bias == 0))
    ln2_triv = bool(np.all(ln2_scale == 1) and np.all(ln2_bias == 0))
    qk_triv = bool(np.all(b_qkv[: 2 * D] == 0))
    apply_c1 = bool(np.any(c1 != 0))
    apply_bfc2 = bool(np.any(bfc2 != 0))

    nc = _get_nc(ln1_triv, ln2_triv, qk_triv, apply_c1, apply_bfc2)

    base = {
        "w_qkv": np.asarray(w_qkv, np.float16),
        "w_proj": np.asarray(w_proj, np.float16),
        "w_fc1": np.asarray(w_fc1, np.float16),
        "w_fc2": np.asarray(w_fc2, np.float16),
        "b_qkv": b_qkv,
        "b_fc1": np.asarray(b_fc1, np.float32),
        "ln1_scale": ln1_scale,
        "ln1_bias": ln1_bias,
        "ln2_scale": ln2_scale,
        "ln2_bias": ln2_bias,
    }
    if apply_c1:
        base["c1"] = c1
    if apply_bfc2:
        base["b_fc2c"] = bfc2

    in_maps = [dict(base, x=np.ascontiguousarray(x[i])) for i in range(B)]
    last_err = None
    for _attempt in range(3):
        try:
            res = run_bass_kernel_spmd(nc, in_maps, core_ids=list(range(B)))
            break
        except Exception as e:  # transient NRT/axon worker failures recover on retry
            last_err = e
            import time as _time

            _time.sleep(2.0)
    else:
        raise last_err
    out = np.stack([res.results[i]["y"] for i in range(B)], axis=0)
    return np.ascontiguousarray(out.astype(np.float32))
